# revision 15
# baseline (speedup 1.0000x reference)
"""Batched Pearson-correlation graph builder on 8 Trainium2 NeuronCores.

Problem: x [128, 64, 30720] f32  ->  (adj [128, 64, 64] f32, x passthrough)
  adj = threshold(|corr| >= 0.5) * corr, zero diagonal, where corr is the
  per-batch Pearson correlation across the 64 channels (over T=30720).

Strategy (pure data parallel, 16 batches per core):
  - Pack 2 batches into the 128 SBUF partitions -> 8 "pairs" per core.
  - One-pass covariance: cov = X@X^T - s s^T / T  (s = row sums), so the
    1 GB input is streamed exactly once from HBM.
  - Per 128-wide k-tile: PE transpose (bf16) -> PSUM, DVE/ACT copy to SBUF,
    then Gram matmul (lhsT = rhs = transposed tile) accumulating into PSUM,
    plus an N=1 matmul against a ones column for the row sums.
  - Epilogue per pair: mean correction via a K=1 outer-product matmul into
    the same PSUM bank, diagonal extraction, rsqrt, outer-product scaling,
    |corr|>=0.5 masking, and DMA of the two 64x64 diagonal blocks.
  - f32 -> bf16 cast happens during the HBM DMA (SWDGE cast-on-load).
"""

import numpy as np

import concourse.bass as bass
import concourse.bacc as bacc
import concourse.mybir as mybir
import concourse.tile as tile
from concourse.bass_utils import run_bass_kernel_spmd
from concourse.masks import make_identity

B, N, T = 128, 64, 30720
N_CORES = 8
BATCH_PER_CORE = B // N_CORES          # 16
ROWS_PER_CORE = BATCH_PER_CORE * N    # 1024
KT = 128                               # k-tile (contraction chunk)
COPY_W = 512                           # PSUM->SBUF copy width (4 k-tiles)
CHUNK = 7680                           # time chunk per DMA load
F32 = mybir.dt.float32
BF16 = mybir.dt.bfloat16


def build_program(rows=ROWS_PER_CORE, t=T, chunk=CHUNK, dma_cast=True):
    assert t % chunk == 0 and chunk % COPY_W == 0 and COPY_W % KT == 0
    n_pairs = rows // 128
    n_chunks = t // chunk
    copies_per_chunk = chunk // COPY_W
    kt_per_copy = COPY_W // KT
    n_ktiles = t // KT

    nc = bacc.Bacc()
    x = nc.declare_dram_parameter("x", [rows, t], F32, isOutput=False)
    adj = nc.declare_dram_parameter(
        "adj", [2 * n_pairs, N, N], F32, isOutput=True
    )

    with tile.TileContext(nc) as tc:
        with (
            tc.tile_pool(name="singles", bufs=1) as singles,
            tc.tile_pool(name="nat", bufs=3) as nat_pool,
            tc.tile_pool(name="xt", bufs=4) as xt_pool,
            tc.tile_pool(name="small", bufs=2) as small_pool,
            tc.tile_pool(name="big", bufs=2) as big_pool,
            tc.tile_pool(name="acc", bufs=2, space="PSUM") as acc_pool,
            tc.tile_pool(name="pt", bufs=3, space="PSUM") as pt_pool,
            tc.tile_pool(name="epi", bufs=2, space="PSUM") as epi_pool,
        ):
            eye_bf = singles.tile([128, 128], BF16)
            make_identity(nc, eye_bf)
            eye_f32 = singles.tile([128, 128], F32)
            make_identity(nc, eye_f32)
            # 1 everywhere except the (per-batch) diagonal
            diagmask = singles.tile([128, 128], F32)
            nc.gpsimd.memset(diagmask, 1.0)
            nc.gpsimd.affine_select(
                out=diagmask,
                in_=diagmask,
                compare_op=mybir.AluOpType.not_equal,
                fill=0.0,
                base=0,
                pattern=[[-1, 128]],
                channel_multiplier=1,
            )
            ones_bf = singles.tile([128, 1], BF16)
            nc.vector.memset(ones_bf, 1.0)
            eps_sb = singles.tile([128, 1], F32)
            nc.vector.memset(eps_sb, 1e-12)

            nat_dt = BF16 if dma_cast else F32

            for p in range(n_pairs):
                # Gram in cols 0:128, row sums in col 128 — one PSUM bank.
                gs_ps = acc_pool.tile([128, 132], F32, tag="g")
                g_ps = gs_ps[:, 0:128]
                s_ps = gs_ps[:, 128:129]
                kidx = 0
                for ch in range(n_chunks):
                    nat = nat_pool.tile([128, chunk], nat_dt)
                    src = x[128 * p : 128 * (p + 1), ch * chunk : (ch + 1) * chunk]
                    if dma_cast:
                        nc.gpsimd.dma_start(out=nat, in_=src)
                    else:
                        nc.sync.dma_start(out=nat, in_=src)
                    for cp in range(copies_per_chunk):
                        pt = pt_pool.tile([128, COPY_W], nat_dt, tag="pt")
                        for j in range(kt_per_copy):
                            nc.tensor.transpose(
                                pt[:, j * KT : (j + 1) * KT],
                                nat[:, cp * COPY_W + j * KT : cp * COPY_W + (j + 1) * KT],
                                eye_bf if nat_dt == BF16 else eye_f32,
                            )
                        xt = xt_pool.tile([128, COPY_W], BF16, tag="xt")
                        if cp % 2 == 0:
                            nc.vector.tensor_copy(xt, pt)
                        else:
                            nc.scalar.activation(
                                xt, pt, mybir.ActivationFunctionType.Copy
                            )
                        for j in range(kt_per_copy):
                            sl = xt[:, j * KT : (j + 1) * KT]
                            nc.tensor.matmul(
                                g_ps, lhsT=sl, rhs=sl,
                                start=(kidx == 0), stop=False,
                                skip_group_check=True,
                            )
                            nc.tensor.matmul(
                                s_ps, lhsT=sl, rhs=ones_bf,
                                start=(kidx == 0), stop=(kidx == n_ktiles - 1),
                                skip_group_check=True,
                            )
                            kidx += 1

                # ---- epilogue: cov correction, normalize, mask ----
                s_sb = small_pool.tile([128, 1], F32, tag="s_sb")
                nc.vector.tensor_copy(s_sb, s_ps)
                sT_ps = epi_pool.tile([1, 128], F32, tag="epi")
                nc.tensor.transpose(sT_ps, s_sb, eye_f32)
                sT_sb = small_pool.tile([1, 128], F32, tag="row")
                nc.vector.tensor_copy(sT_sb, sT_ps)
                sTn_sb = small_pool.tile([1, 128], F32, tag="rown")
                nc.vector.tensor_scalar_mul(sTn_sb, sT_sb, -1.0 / t)
                # cov = G - s s^T / T, accumulated straight into the G bank
                nc.tensor.matmul(
                    g_ps, lhsT=sTn_sb, rhs=sT_sb,
                    start=False, stop=True, skip_group_check=True,
                )

                # diag(cov) -> norms^2
                junk = big_pool.tile([128, 128], F32, tag="junk")
                nc.vector.tensor_mul(junk, g_ps, eye_f32)
                diag_sb = small_pool.tile([128, 1], F32, tag="diag")
                nc.vector.tensor_reduce(
                    diag_sb, junk, axis=mybir.AxisListType.X,
                    op=mybir.AluOpType.add,
                )
                # inv = 1/sqrt(diag)
                nc.scalar.activation(
                    diag_sb, diag_sb, mybir.ActivationFunctionType.Sqrt,
                    bias=eps_sb,
                )
                inv_sb = small_pool.tile([128, 1], F32, tag="inv")
                nc.vector.reciprocal(inv_sb, diag_sb)
                iT_ps = epi_pool.tile([1, 128], F32, tag="epi")
                nc.tensor.transpose(iT_ps, inv_sb, eye_f32)
                iT_sb = small_pool.tile([1, 128], F32, tag="row2")
                nc.vector.tensor_copy(iT_sb, iT_ps)
                # R = outer(inv, inv), then zero the diagonal
                r_ps = epi_pool.tile([128, 128], F32, tag="epi")
                nc.tensor.matmul(r_ps, lhsT=iT_sb, rhs=iT_sb, start=True, stop=True)
                rz_sb = big_pool.tile([128, 128], F32, tag="rz")
                nc.vector.tensor_mul(rz_sb, r_ps, diagmask)
                corr_sb = big_pool.tile([128, 128], F32, tag="corr")
                nc.vector.tensor_mul(corr_sb, g_ps, rz_sb)
                # mask = (|corr| >= 0.5), adj = corr * mask
                abs_sb = big_pool.tile([128, 128], F32, tag="abs")
                nc.scalar.activation(
                    abs_sb, corr_sb, mybir.ActivationFunctionType.Abs
                )
                mask_sb = big_pool.tile([128, 128], F32, tag="mask")
                nc.vector.tensor_scalar(
                    out=mask_sb, in0=abs_sb,
                    scalar1=0.5, scalar2=None,
                    op0=mybir.AluOpType.is_ge,
                )
                adj_sb = big_pool.tile([128, 128], F32, tag="adj")
                nc.vector.tensor_mul(adj_sb, corr_sb, mask_sb)
                nc.sync.dma_start(out=adj[2 * p], in_=adj_sb[0:64, 0:64])
                nc.sync.dma_start(out=adj[2 * p + 1], in_=adj_sb[64:128, 64:128])

    if not nc.is_finalized():
        nc.finalize()
    return nc


def kernel(x):
    x = np.asarray(x)
    assert x.shape == (B, N, T), x.shape
    x32 = np.ascontiguousarray(x, dtype=np.float32)
    shards = x32.reshape(N_CORES, ROWS_PER_CORE, T)
    nc = build_program()
    in_maps = [{"x": shards[i]} for i in range(N_CORES)]
    res = run_bass_kernel_spmd(nc, in_maps, list(range(N_CORES)))
    adj = np.concatenate(
        [res.results[i]["adj"].reshape(BATCH_PER_CORE, N, N) for i in range(N_CORES)],
        axis=0,
    ).astype(np.float32)
    return adj, x
